# revision 16
# baseline (speedup 1.0000x reference)
"""Bass/Trainium2 kernel for nn_MAC_30554397344312 (gnn_message_passing).

Reference computation (B=256 rollout groups, n=64 agents, D=256):
    comm = h @ W_act.T + b_act                      # (B*n, D)
    agg[b,j] = sum_i mask[i,j] * comm[b,i] / (n-1)  # mask = ones - eye
    x   = agg @ W_sum.T + b_sum
    out = relu(x @ W_head.T + b_head)

Everything before the relu is linear, so fold on host:
    Wc = W_head @ W_sum @ W_act          (256x256)
    bc = b_head + b_sum @ W_head.T + b_act @ (W_head @ W_sum).T
    out[b,j] = relu( (A @ H_b)[j] @ Wc.T + bc ),  A = (ones-eye)/(n-1)

On device (per core, 2048 rows = 32 agent-groups):
    stage 1 (PE): Y.T tiles [d, tok] via matmul(lhsT=H_tile[128tok,128d],
                  rhs=blockdiag(A,A)) - aggregation and transpose fused.
    stage 2 (DVE): evict Y.T PSUM banks to SBUF.
    stage 3 (PE): out[tok, d_out] = Y.T.T @ Wc.T accumulated over 2 k-chunks.
    stage 4 (ACT): relu + PSUM->SBUF evict.  stage 5: contiguous DMA store.

Sharding: data-parallel over the B axis, 8 cores x 32 groups.
"""

from contextlib import ExitStack

import numpy as np

import concourse.bacc as bacc
import concourse.bass as bass
import concourse.tile as tile
from concourse import mybir
from concourse.bass_utils import run_bass_kernel_spmd

N_AGENTS = 64
B = 256
D = 256
N_CORES = 8
ROWS = B * N_AGENTS            # 16384
ROWS_PER_CORE = ROWS // N_CORES  # 2048
P = 128
N_TILES = ROWS_PER_CORE // P   # 16 token tiles per core
TILES_PER_CHUNK = 4            # DMA granularity: 512 KiB
N_CHUNKS = N_TILES // TILES_PER_CHUNK
W_SCALE = 16.0  # fp16 weight prescale (power of 2; inverted exactly in relu)

_cache = {}


def _build(has_bias: bool, f16: bool = True):
    f32 = mybir.dt.float32
    mdt = mybir.dt.float16 if f16 else mybir.dt.float32
    inv_scale = 1.0 / W_SCALE if f16 else 1.0
    nc = bacc.Bacc("TRN2", target_bir_lowering=False, debug=False,
                   num_devices=N_CORES)

    h = nc.dram_tensor("h", [ROWS_PER_CORE, D], f32, kind="ExternalInput")
    wcT = nc.dram_tensor("wcT", [D, D], mdt, kind="ExternalInput")
    ablk = nc.dram_tensor("ablk", [P, P], mdt, kind="ExternalInput")
    if has_bias:
        bc = nc.dram_tensor("bc", [1, D], f32, kind="ExternalInput")
    out = nc.dram_tensor("out", [ROWS_PER_CORE, D], f32, kind="ExternalOutput")

    h_ap = h[:, :].rearrange("(n p) d -> p n d", p=P)      # [128, 16, 256]
    out_ap = out[:, :].rearrange("(n p) d -> p n d", p=P)  # [128, 16, 256]

    with tile.TileContext(nc) as tc:
        with ExitStack() as ctx:
            const = ctx.enter_context(tc.tile_pool(name="const", bufs=1))
            ytps = ctx.enter_context(
                tc.tile_pool(name="ytps", bufs=3, space="PSUM"))
            outps = ctx.enter_context(
                tc.tile_pool(name="outps", bufs=4, space="PSUM"))

            a_t = const.tile([P, P], mdt, tag="a", name="a_t")
            nc.sync.dma_start(out=a_t[:], in_=ablk[:, :])
            w_t = [const.tile([P, D], mdt, tag=f"w{k}", name=f"w_{k}") for k in range(2)]
            if has_bias:
                bc_t = const.tile([P, D], f32, tag="bc", name="bc_t")
                bc_bcast = bass.AP(
                    tensor=bc, offset=0, ap=[[0, P], [1, D]])
                nc.gpsimd.dma_start(out=bc_t[:], in_=bc_bcast)

            # h loads: f32 in 8 chunks of 2 tiles (256 KiB), alternating
            # between the two HWDGE rings (SP=sync, ACT=scalar) so the two
            # streams move in parallel; DVE casts f32 -> fp16 for the PE.
            htiles = []  # 16 per-token-tile fp16 APs [128, 256]
            LC = 2  # tiles per load chunk
            for c in range(N_TILES // LC):
                traw = const.tile([P, LC, D], f32, tag=f"hr{c}", name=f"hr_{c}")
                eng = nc.sync if c % 2 == 0 else nc.scalar
                eng.dma_start(
                    out=traw[:], in_=h_ap[:, c * LC:(c + 1) * LC, :])
                if f16:
                    t = const.tile([P, LC, D], mdt, tag=f"hc{c}", name=f"hc_{c}")
                    nc.vector.tensor_copy(out=t[:], in_=traw[:])
                else:
                    t = traw
                htiles.extend(t[:, s, :] for s in range(LC))
            # weights load after the first h chunks are queued (needed later)
            for k in range(2):
                nc.sync.dma_start(out=w_t[k][:], in_=wcT[k * P:(k + 1) * P, :])

            # Y.T in SBUF: two d-chunks, each [128 d, 2048 tok]
            yt = [const.tile([P, ROWS_PER_CORE], mdt, tag=f"yt{k}", name=f"yt_{k}")
                  for k in range(2)]
            och = [const.tile([P, TILES_PER_CHUNK, D], f32, tag=f"oc{c}", name=f"oc_{c}")
                   for c in range(N_CHUNKS)]

            # ---- PE stream: agg batches interleaved with main batches so
            # the PE has main-matmul work while waiting on late h chunks ----
            def agg_batch(b):
                # one PSUM bank per (b, k): token tiles 4b..4b+3
                ps = [ytps.tile([P, TILES_PER_CHUNK * P], f32, tag="ytps",
                                name="yt_ps") for _ in range(2)]
                for s in range(TILES_PER_CHUNK):
                    for k in range(2):
                        lhsT = htiles[b * TILES_PER_CHUNK + s][:, k * P:(k + 1) * P]
                        nc.tensor.matmul(
                            ps[k][:, s * P:(s + 1) * P], lhsT, a_t[:],
                            start=True, stop=True)
                for k in range(2):
                    nc.vector.tensor_copy(
                        yt[k][:, b * TILES_PER_CHUNK * P:(b + 1) * TILES_PER_CHUNK * P],
                        ps[k][:])

            def main_batch(c):
                for s in range(TILES_PER_CHUNK):
                    m = c * TILES_PER_CHUNK + s
                    po = outps.tile([P, D], f32, tag="outps", name="po")
                    for k in range(2):
                        nc.tensor.matmul(
                            po[:], yt[k][:, m * P:(m + 1) * P], w_t[k][:],
                            start=(k == 0), stop=(k == 1))
                    dst = och[c][:, s, :]
                    if has_bias:
                        nc.vector.tensor_scalar(
                            out=dst, in0=po[:], scalar1=inv_scale, scalar2=None,
                            op0=mybir.AluOpType.mult)
                        nc.vector.tensor_tensor(
                            out=dst, in0=dst, in1=bc_t[:],
                            op=mybir.AluOpType.add)
                        nc.scalar.activation(
                            out=dst, in_=dst,
                            func=mybir.ActivationFunctionType.Relu)
                    elif m % 2 == 0:
                        nc.scalar.activation(
                            out=dst, in_=po[:],
                            func=mybir.ActivationFunctionType.Relu,
                            scale=inv_scale)
                    else:
                        nc.vector.tensor_scalar(
                            out=dst, in0=po[:], scalar1=inv_scale,
                            scalar2=0.0, op0=mybir.AluOpType.mult,
                            op1=mybir.AluOpType.max)
                (nc.sync if c % 2 == 0 else nc.scalar).dma_start(
                    out=out_ap[:, c * TILES_PER_CHUNK:(c + 1) * TILES_PER_CHUNK, :],
                    in_=och[c][:])

            agg_batch(0)
            agg_batch(1)
            main_batch(0)
            agg_batch(2)
            main_batch(1)
            agg_batch(3)
            main_batch(2)
            main_batch(3)
    nc.finalize()
    return nc


def _fold(W_act, b_act, W_sum, b_sum, W_head, b_head, f16=True):
    Wa = W_act.astype(np.float64)
    Ws = W_sum.astype(np.float64)
    Wh = W_head.astype(np.float64)
    Wc = Wh @ Ws @ Wa
    bc = (b_head.astype(np.float64)
          + b_sum.astype(np.float64) @ Wh.T
          + b_act.astype(np.float64) @ (Wh @ Ws).T)
    A = np.ones((N_AGENTS, N_AGENTS)) - np.eye(N_AGENTS)
    if f16:
        # mask stays exact 0/1 in fp16; 1/63 and the fp16-subnormal
        # prescale fold into the weights, inverted via the relu scale.
        WcT = (Wc.T / (N_AGENTS - 1) * W_SCALE).astype(np.float16)
        wdt = np.float16
    else:
        A = A / (N_AGENTS - 1)
        WcT = Wc.T.astype(np.float32)
        wdt = np.float32
    Ablk = np.zeros((P, P))
    Ablk[:N_AGENTS, :N_AGENTS] = A
    Ablk[N_AGENTS:, N_AGENTS:] = A
    return (np.ascontiguousarray(WcT), bc.astype(np.float32),
            Ablk.astype(wdt))


def kernel(hidden_state, W_act, b_act, W_sum, b_sum, W_head, b_head,
           _trace=False, _tmpdir=None):
    import os
    f16 = os.environ.get("KERNEL_F32", "0") != "1"
    h = np.ascontiguousarray(np.asarray(hidden_state, dtype=np.float32))
    WcT, bc, Ablk = _fold(np.asarray(W_act), np.asarray(b_act),
                          np.asarray(W_sum), np.asarray(b_sum),
                          np.asarray(W_head), np.asarray(b_head), f16=f16)
    has_bias = bool(np.any(bc))
    if (has_bias, f16) not in _cache:
        _cache[(has_bias, f16)] = _build(has_bias, f16=f16)
    nc = _cache[(has_bias, f16)]

    in_maps = []
    for c in range(N_CORES):
        m = {"h": h[c * ROWS_PER_CORE:(c + 1) * ROWS_PER_CORE],
             "wcT": WcT, "ablk": Ablk}
        if has_bias:
            m["bc"] = bc.reshape(1, D)
        in_maps.append(m)

    res = run_bass_kernel_spmd(
        nc, in_maps, core_ids=list(range(N_CORES)),
        trace=_trace, tmpdir=_tmpdir)
    out = np.concatenate([res.results[c]["out"] for c in range(N_CORES)],
                         axis=0)
    if _trace:
        return out, res
    return out
